# revision 1
# baseline (speedup 1.0000x reference)
"""Trainium2 Bass kernel for nn_DecoderBlock (B=8, T=TE=1024, H=1024, NH=8).

Strategy: pure data-parallel over batch — batch element b runs on NeuronCore b,
no collectives. All on-chip compute is done in transposed layout [feature,
token] so no on-chip transposes are ever needed:
  - host pre-transposes input_/encoder_output and all weight matrices
  - layernorm stats (sums over the feature axis = partition axis) via
    ones-vector matmuls on the PE; affine params become per-partition scalars
  - attention computes scores transposed (s^T[k,q] = K^T-block^T... i.e.
    lhsT=K^T, rhs=Q^T), softmax denominator is folded in after the context
    matmul (exp without max-subtraction is safe: |scores| <~ 8 here)
  - causal-mask blocks that are fully masked are skipped entirely; partially
    masked blocks multiply exp(s) by a 0/1 mask tile loaded from the host
Matmuls in bf16 with fp32 PSUM accumulation; residual stream kept in fp32.
"""

import sys

for _p in ("/opt/trn_rl_repo", "/root/.axon_site/_ro/trn_rl_repo"):
    if _p not in sys.path:
        sys.path.append(_p)

import numpy as np
import ml_dtypes

import concourse.bass as bass
import concourse.mybir as mybir
import concourse.tile as tile
from concourse import bacc

BF16 = ml_dtypes.bfloat16
F32 = mybir.dt.float32
F16 = mybir.dt.float16
BF = mybir.dt.bfloat16

B = 8
T = 1024
TE = 1024
H = 1024
NH = 8
DK = H // NH  # 128
FF = 4 * H
P = 128
NT = H // P       # 8 feature blocks
NTK = T // P      # 8 key blocks
NQ = 2            # token chunks
QW = T // NQ      # 512
NCORES = 8
EPS = 1e-5
ISCALE = float(1.0 / np.sqrt(DK))

FULL, MASKED, SKIP = 0, 1, 2

AOP = mybir.AluOpType
AF = mybir.ActivationFunctionType

_cache = {}


def _classify(mask):
    """mask: [B, TQ, TK] bool (True = masked out). Block structure over
    (k_block, q_chunk), unioned across batch so one NEFF serves all cores."""
    cls = np.zeros((NTK, NQ), np.int32)
    for kb in range(NTK):
        for qc in range(NQ):
            blk = mask[:, qc * QW:(qc + 1) * QW, kb * P:(kb + 1) * P]
            if blk.all():
                cls[kb, qc] = SKIP
            elif blk.any():
                cls[kb, qc] = MASKED
            else:
                cls[kb, qc] = FULL
    return cls


def _build(cls_self, cls_cross, canon_s=None, canon_c=None, reps=1):
    nc = bacc.Bacc("TRN2", target_bir_lowering=False, debug=False,
                   num_devices=NCORES)

    xT_d = nc.dram_tensor("xT", [H, T], F32, kind="ExternalInput")
    encT_d = nc.dram_tensor("encT", [H, TE], BF, kind="ExternalInput")
    mm_s_d = nc.dram_tensor("mm_s", [T, T], BF, kind="ExternalInput")
    mm_c_d = nc.dram_tensor("mm_c", [TE, T], BF, kind="ExternalInput")
    wd = {}
    for nm in ("wq_s", "wk_s", "wv_s", "wo_s", "wq_c", "wk_c", "wv_c", "wo_c"):
        wd[nm] = nc.dram_tensor(nm, [H, H], BF, kind="ExternalInput")
    w1T_d = nc.dram_tensor("w1T", [H, FF], BF, kind="ExternalInput")
    w2T_d = nc.dram_tensor("w2T", [FF, H], BF, kind="ExternalInput")
    vd = {}
    vd["b1"] = nc.dram_tensor("b1", [FF], F32, kind="ExternalInput")
    for nm in ("b2", "g1", "bb1", "g2", "bb2", "g3", "bb3"):
        vd[nm] = nc.dram_tensor(nm, [H], F32, kind="ExternalInput")
    outT_d = nc.dram_tensor("outT", [H, T], F32, kind="ExternalOutput")

    with tile.TileContext(nc) as tc:
        for _ in range(reps):
            _emit(nc, tc, cls_self, cls_cross, canon_s, canon_c,
                  xT_d, encT_d, mm_s_d, mm_c_d,
                  wd, w1T_d, w2T_d, vd, outT_d)
    nc.compile()
    return nc


def _emit(nc, tc, cls_self, cls_cross, canon_s, canon_c,
          xT_d, encT_d, mm_s_d, mm_c_d,
          wd, w1T_d, w2T_d, vd, outT_d):

    def canon_key_fn(dname, kb, qc):
        cmap = canon_s if dname == "mm_s" else canon_c
        if cmap is None:
            return f"{kb}_{qc}"
        return cmap[(kb, qc)]
    import contextlib
    ctx = contextlib.ExitStack()
    with ctx:
        # f16 is used only for softmax-denominator / LN-stat broadcast
        # intermediates where ~5e-4 relative error is acceptable by design.
        ctx.enter_context(nc.allow_low_precision(
            reason="f16 broadcast/denominator intermediates"))
        persist = ctx.enter_context(tc.tile_pool(name="persist", bufs=1))
        bigs = ctx.enter_context(tc.tile_pool(name="bigs", bufs=1))
        wpool = ctx.enter_context(tc.tile_pool(name="wpool", bufs=2))
        epool = ctx.enter_context(tc.tile_pool(name="epool", bufs=2))
        accp = ctx.enter_context(tc.tile_pool(name="accp", bufs=2))
        tmpp = ctx.enter_context(tc.tile_pool(name="tmpp", bufs=2))
        smp = ctx.enter_context(tc.tile_pool(name="smp", bufs=1))
        rdp = ctx.enter_context(tc.tile_pool(name="rdp", bufs=2))
        stg = ctx.enter_context(tc.tile_pool(name="stg", bufs=2))

        # ---- constants / params ----
        ones_k = persist.tile([P, 1], F32, tag="ones_k", name="ones_k")
        nc.vector.memset(ones_k, 1.0)
        ones_kb = persist.tile([P, 1], BF, tag="ones_kb", name="ones_kb")
        nc.vector.memset(ones_kb, 1.0)
        ones_k16 = persist.tile([P, 1], F16, tag="ones_k16", name="ones_k16")
        nc.vector.memset(ones_k16, 1.0)
        ones_r16 = persist.tile([1, P], F16, tag="ones_r16", name="ones_r16")
        nc.vector.memset(ones_r16, 1.0)
        ones_r = persist.tile([1, P], F32, tag="ones_r", name="ones_r")
        nc.vector.memset(ones_r, 1.0)
        eps_t = persist.tile([1, 1], F32, tag="eps", name="eps")
        nc.vector.memset(eps_t, EPS)

        # ---- residual stream x^T in fp32 ----
        xres = []
        for k in range(NT):
            t = persist.tile([P, T], F32, tag=f"xres{k}", name=f"xres{k}")
            xres.append(t)
        for c in range(NQ):
            for k in range(NT):
                nc.sync.dma_start(
                    out=xres[k][:, c * QW:(c + 1) * QW],
                    in_=xT_d.ap()[k * P:(k + 1) * P, c * QW:(c + 1) * QW])

        def load_vec(name, n):
            t = persist.tile([P, n // P], F32, tag=f"v_{name}", name=f"v_{name}")
            nc.sync.dma_start(out=t, in_=vd[name].ap().rearrange(
                "(n p) -> p n", p=P))
            return t

        g1 = load_vec("g1", H); bb1 = load_vec("bb1", H)
        g2 = load_vec("g2", H); bb2 = load_vec("bb2", H)
        g3 = load_vec("g3", H); bb3 = load_vec("bb3", H)
        b1 = load_vec("b1", FF); b2 = load_vec("b2", H)

        # mask multiplier tiles for partially-masked blocks; blocks whose
        # content is identical across (kb, qc) (e.g. causal diagonals) share
        # one SBUF tile, keyed by the canonical block in cls (negative codes).
        mtiles_s, mtiles_c = {}, {}
        for (cls, dram, store) in ((cls_self, mm_s_d, mtiles_s),
                                   (cls_cross, mm_c_d, mtiles_c)):
            canon = {}
            for kb in range(NTK):
                for qc in range(NQ):
                    if cls[kb, qc] != MASKED:
                        continue
                    key = canon_key_fn(dram.name, kb, qc)
                    if key not in canon:
                        mt = persist.tile([P, QW], BF,
                                          tag=f"msk_{dram.name}_{key}",
                                          name=f"msk_{dram.name}_{key}")
                        nc.sync.dma_start(
                            out=mt,
                            in_=dram.ap()[kb * P:(kb + 1) * P,
                                          qc * QW:(qc + 1) * QW])
                        canon[key] = mt
                    store[(kb, qc)] = canon[key]

        # big bf16 [P, T] tile groups (tags only; allocation at write time)
        def big(group, j):
            return bigs.tile([P, T], BF, tag=f"big{group}{j}", name=f"big{group}{j}")

        # ---------- helpers ----------
        def layer_norm(src_tiles, g, bb, gidx, dst_group):
            """src: 8 fp32 [P,T] tiles; returns 8 bf16 [P,T] tiles (dst_group)."""
            dst = [None] * NT
            with tc.tile_pool(name=f"ln{gidx}", bufs=1, space="PSUM", side="left") as pp:
                for c in range(NQ):
                    sl = slice(c * QW, (c + 1) * QW)
                    ps_sx = pp.tile([1, QW], F32, tag="sx", name="sx")
                    ps_sq = pp.tile([1, QW], F32, tag="sq", name="sq")
                    for k in range(NT):
                        xb = stg.tile([P, QW], BF, tag="xb", name="xb")
                        nc.vector.tensor_copy(out=xb, in_=src_tiles[k][:, sl])
                        sq = stg.tile([P, QW], BF, tag="sqt", name="sqt")
                        nc.vector.tensor_mul(out=sq, in0=xb, in1=xb)
                        nc.tensor.matmul(ps_sx, lhsT=ones_kb, rhs=xb,
                                         start=(k == 0), stop=(k == NT - 1))
                        nc.tensor.matmul(ps_sq, lhsT=ones_kb, rhs=sq,
                                         start=(k == 0), stop=(k == NT - 1))
                    mu = smp.tile([1, QW], F16, tag="mu", name="mu")
                    m2 = smp.tile([1, QW], F32, tag="m2", name="m2")
                    rs = smp.tile([1, QW], F16, tag="rs", name="rs")
                    nc.scalar.mul(out=mu, in_=ps_sx, mul=1.0 / H)
                    nc.scalar.mul(out=m2, in_=ps_sq, mul=1.0 / H)
                    # rs doubles as mu^2 scratch before holding 1/std
                    nc.vector.tensor_mul(out=rs, in0=mu, in1=mu)
                    nc.vector.tensor_sub(out=m2, in0=m2, in1=rs)
                    # m2 := sqrt(var + eps)
                    nc.scalar.activation(out=m2, in_=m2, func=AF.Sqrt,
                                         bias=eps_t)
                    nc.vector.reciprocal(out=rs, in_=m2)
                    ps_bm = pp.tile([P, QW], F32, tag="bm", name="bm")
                    ps_br = pp.tile([P, QW], F32, tag="br", name="br")
                    nc.tensor.matmul(ps_bm, lhsT=ones_r16, rhs=mu,
                                     start=True, stop=True)
                    nc.tensor.matmul(ps_br, lhsT=ones_r16, rhs=rs,
                                     start=True, stop=True)
                    # broadcasts to SBUF once per chunk so the per-tile DVE
                    # ops run in 2x mode (PSUM operands force 1x)
                    bm = tmpp.tile([P, QW], F32, tag="bm_sb", name="bm_sb", bufs=1)
                    nc.vector.tensor_copy(out=bm, in_=ps_bm)
                    br = tmpp.tile([P, QW], F32, tag="br_sb", name="br_sb", bufs=1)
                    nc.vector.tensor_copy(out=br, in_=ps_br)
                    for k in range(NT):
                        if dst[k] is None and c == 0:
                            dst[k] = big(dst_group, k)
                        tmp = tmpp.tile([P, QW], F32, tag="lnt", name="lnt")
                        nc.vector.tensor_sub(out=tmp, in0=src_tiles[k][:, sl],
                                             in1=bm)
                        nc.vector.tensor_mul(out=tmp, in0=tmp, in1=br)
                        nc.vector.tensor_scalar(
                            out=dst[k][:, sl], in0=tmp,
                            scalar1=g[:, k:k + 1], scalar2=bb[:, k:k + 1],
                            op0=AOP.mult, op1=AOP.add)
            return dst

        def load_w(dram, colsl=None, tag_suffix=""):
            """Load [H or P*8, 1024-wide] weight block-rows into wpool tags."""
            tiles = []
            for k in range(NT):
                t = wpool.tile([P, H], BF, tag=f"w{k}", name=f"w{k}")
                src = dram.ap()[k * P:(k + 1) * P, :] if colsl is None else \
                    dram.ap()[k * P:(k + 1) * P, colsl]
                nc.sync.dma_start(out=t, in_=src)
                tiles.append(t)
            return tiles

        def proj_T(src_tiles, wname, dst_group, pp):
            """dst^T[m][:,c] = sum_k W^T[k,m-block]^T... out = W @ src^T.
            Returns 8 bf16 [P,T] tiles."""
            wt = load_w(wd[wname])
            dst = []
            for m in range(NT):
                d = big(dst_group, m)
                pss = [pp.tile([P, QW], F32, tag=f"pp{c}", name=f"pp{c}")
                       for c in range(NQ)]
                for k in range(NT):
                    for c in range(NQ):
                        nc.tensor.matmul(pss[c],
                                         lhsT=wt[k][:, m * P:(m + 1) * P],
                                         rhs=src_tiles[k][:, c * QW:(c + 1) * QW],
                                         start=(k == 0), stop=(k == NT - 1))
                for c in range(NQ):
                    nc.scalar.copy(out=d[:, c * QW:(c + 1) * QW], in_=pss[c])
                dst.append(d)
            return dst

        def proj_nat(src_tiles, wname, dst_group, pp):
            """V = src @ W.T in natural [token, feature] layout."""
            wt = load_w(wd[wname])
            dst = []
            for tb in range(NT):
                d = big(dst_group, tb)
                pss = [pp.tile([P, QW], F32, tag=f"pp{c}", name=f"pp{c}")
                       for c in range(NQ)]
                for k in range(NT):
                    for c in range(NQ):
                        nc.tensor.matmul(pss[c],
                                         lhsT=src_tiles[k][:, tb * P:(tb + 1) * P],
                                         rhs=wt[k][:, c * QW:(c + 1) * QW],
                                         start=(k == 0), stop=(k == NT - 1))
                for c in range(NQ):
                    nc.scalar.copy(out=d[:, c * QW:(c + 1) * QW], in_=pss[c])
                dst.append(d)
            return dst

        def attention(qT, kT, v, cls, mtiles, dst_group):
            """qT,kT: 8 [P(d),T] bf16 tiles (tile h = head h); v: 8 [P(t),H]
            bf16 tiles. Returns c^T as 8 bf16 [P,T] tiles (tile h = head h).

            Software-pipelined over (head, chunk) units: unit i+1's scores
            matmuls are emitted before unit i's den/bcast/ctx matmuls so the
            PE has work while unit i's softmax (ACT exp + DVE tree) runs."""
            cT = {}
            units = [(h, qc) for h in range(NH) for qc in range(NQ)]

            def stage1(i, h, qc, pp):
                """paired scores -> exp -> masked mul -> denominator reduce."""
                qsl = slice(qc * QW, (qc + 1) * QW)
                kbs = [kb for kb in range(NTK) if cls[kb, qc] != SKIP]
                n = len(kbs)
                eall = epool.tile([P, NTK, QW], BF, tag="eall", name="eall")
                idx = 0
                pi = 0
                while idx < n:
                    m = min(2, n - idx)
                    ps = pp.tile([P, 2 * QW], F32, tag=f"s{pi % 2}",
                                 name=f"s{pi % 2}")
                    for j in range(m):
                        kb = kbs[idx + j]
                        nc.tensor.matmul(
                            ps[:, j * QW:(j + 1) * QW],
                            lhsT=kT[h][:, kb * P:(kb + 1) * P],
                            rhs=qT[h][:, qsl], start=True, stop=True)
                    nc.scalar.activation(
                        out=eall[:, idx:idx + m, :].rearrange("p a b -> p (a b)"),
                        in_=ps[:, 0:m * QW], func=AF.Exp, scale=ISCALE)
                    for j in range(m):
                        kb = kbs[idx + j]
                        if cls[kb, qc] == MASKED:
                            nc.vector.tensor_mul(
                                out=eall[:, idx + j, :], in0=eall[:, idx + j, :],
                                in1=mtiles[(kb, qc)])
                    idx += m
                    pi += 1
                # denominator: sum exp-pair outputs progressively so the
                # tree starts after the SECOND exp, not the last one. Fast
                # path for the even pair counts that actually occur (n=4,8);
                # generic fold otherwise.
                acc = accp.tile([P, QW], F16, tag="acc", name="acc")
                def flat(ap):
                    return ap.rearrange("p a b -> p (a b)")
                if n == 8:
                    pA = accp.tile([P, 2, QW], F16, tag="pA", name="pA")
                    nc.vector.tensor_add(out=flat(pA), in0=flat(eall[:, 0:2, :]),
                                         in1=flat(eall[:, 2:4, :]))
                    pB = accp.tile([P, 2, QW], F16, tag="pB", name="pB")
                    nc.vector.tensor_add(out=flat(pB), in0=flat(eall[:, 4:6, :]),
                                         in1=flat(eall[:, 6:8, :]))
                    nc.vector.tensor_add(out=pA[:, 0, :], in0=pA[:, 0, :],
                                         in1=pA[:, 1, :])
                    nc.vector.tensor_add(out=pB[:, 0, :], in0=pB[:, 0, :],
                                         in1=pB[:, 1, :])
                    nc.vector.tensor_add(out=acc, in0=pA[:, 0, :],
                                         in1=pB[:, 0, :])
                elif n == 4:
                    pA = accp.tile([P, 2, QW], F16, tag="pA", name="pA")
                    nc.vector.tensor_add(out=flat(pA), in0=flat(eall[:, 0:2, :]),
                                         in1=flat(eall[:, 2:4, :]))
                    nc.vector.tensor_add(out=acc, in0=pA[:, 0, :],
                                         in1=pA[:, 1, :])
                else:
                    # generic fold for arbitrary mask structures
                    m = n // 2
                    if m == 1:
                        nc.vector.tensor_add(out=acc, in0=eall[:, 0, :],
                                             in1=eall[:, 1, :])
                        if n % 2:
                            nc.vector.tensor_add(out=acc, in0=acc,
                                                 in1=eall[:, n - 1, :])
                        return kbs, eall, acc
                    a4 = accp.tile([P, NTK // 2, QW], F16, tag="a4",
                                   name="a4", bufs=1)
                    nc.vector.tensor_add(
                        out=flat(a4[:, 0:m, :]), in0=flat(eall[:, 0:m, :]),
                        in1=flat(eall[:, m:2 * m, :]))
                    if n % 2:
                        nc.vector.tensor_add(out=a4[:, 0, :], in0=a4[:, 0, :],
                                             in1=eall[:, n - 1, :])
                    while m > 2:
                        h2 = m // 2
                        nc.vector.tensor_add(
                            out=flat(a4[:, 0:h2, :]), in0=flat(a4[:, 0:h2, :]),
                            in1=flat(a4[:, h2:2 * h2, :]))
                        if m % 2:
                            nc.vector.tensor_add(out=a4[:, 0, :],
                                                 in0=a4[:, 0, :],
                                                 in1=a4[:, m - 1, :])
                        m = h2
                    nc.vector.tensor_add(out=acc, in0=a4[:, 0, :],
                                         in1=a4[:, 1, :])
                return kbs, eall, acc
                a4 = accp.tile([P, NTK // 2, QW], F16, tag="a4", name="a4",
                               bufs=1)
                nc.vector.tensor_add(
                    out=a4[:, 0:m, :].rearrange("p a b -> p (a b)"),
                    in0=eall[:, 0:m, :].rearrange("p a b -> p (a b)"),
                    in1=eall[:, m:2 * m, :].rearrange("p a b -> p (a b)"))
                if n % 2:
                    nc.vector.tensor_add(out=a4[:, 0, :], in0=a4[:, 0, :],
                                         in1=eall[:, n - 1, :])
                while m > 2:
                    h2 = m // 2
                    nc.vector.tensor_add(
                        out=a4[:, 0:h2, :].rearrange("p a b -> p (a b)"),
                        in0=a4[:, 0:h2, :].rearrange("p a b -> p (a b)"),
                        in1=a4[:, h2:2 * h2, :].rearrange("p a b -> p (a b)"))
                    if m % 2:
                        nc.vector.tensor_add(out=a4[:, 0, :], in0=a4[:, 0, :],
                                             in1=a4[:, m - 1, :])
                    m = h2
                nc.vector.tensor_add(out=acc, in0=a4[:, 0, :], in1=a4[:, 1, :])
                return kbs, eall, acc

            def stage2(i, h, qc, kbs, eall, acc, pp):
                """den matmul -> recip -> bcast -> ctx -> cT mul."""
                qsl = slice(qc * QW, (qc + 1) * QW)
                # ctx matmuls first: they need only the e tiles, which are
                # ready well before the denominator tree finishes
                ps_u = pp.tile([P, QW], F32, tag=f"u{qc % 2}",
                               name=f"u{qc % 2}")
                for j, kb in enumerate(kbs):
                    nc.tensor.matmul(
                        ps_u, lhsT=v[kb][:, h * P:(h + 1) * P],
                        rhs=eall[:, j, :],
                        start=(j == 0), stop=(j == len(kbs) - 1))
                ps_den = pp.tile([1, QW], F32, tag=f"db{i % 2}",
                                 name=f"db{i % 2}")
                nc.tensor.matmul(ps_den, lhsT=ones_k16, rhs=acc,
                                 start=True, stop=True)
                rden = rdp.tile([1, QW], F16, tag="rden", name="rden")
                nc.vector.reciprocal(out=rden, in_=ps_den)
                ps_bc = pp.tile([P, QW], F32, tag=f"db{i % 2}",
                                name=f"db{i % 2}b")
                nc.tensor.matmul(ps_bc, lhsT=ones_r16, rhs=rden,
                                 start=True, stop=True)
                brden = rdp.tile([P, QW], F32, tag="brden", name="brden")
                nc.vector.tensor_copy(out=brden, in_=ps_bc)
                if h not in cT:
                    cT[h] = big(dst_group, h)
                nc.vector.tensor_mul(out=cT[h][:, qsl], in0=ps_u, in1=brden)

            with tc.tile_pool(name=f"att{dst_group}", bufs=1,
                              space="PSUM") as pp:
                pending = []
                for i, (h, qc) in enumerate(units):
                    pending.append((i, h, qc) + stage1(i, h, qc, pp))
                    if len(pending) > 1:
                        stage2(*pending.pop(0), pp)
                for item in pending:
                    stage2(*item, pp)
            return [cT[h] for h in range(NH)]

        def out_proj_residual(cT, wname, pp):
            wt = load_w(wd[wname])
            for m in range(NT):
                pss = [pp.tile([P, QW], F32, tag=f"pp{c}", name=f"pp{c}")
                       for c in range(NQ)]
                for k in range(NT):
                    for c in range(NQ):
                        nc.tensor.matmul(pss[c],
                                         lhsT=wt[k][:, m * P:(m + 1) * P],
                                         rhs=cT[k][:, c * QW:(c + 1) * QW],
                                         start=(k == 0), stop=(k == NT - 1))
                for c in range(NQ):
                    sl = slice(c * QW, (c + 1) * QW)
                    nc.vector.tensor_add(out=xres[m][:, sl],
                                         in0=xres[m][:, sl], in1=pss[c])

        # ================= phases =================
        # LN1 + self-attention
        xn = layer_norm(xres, g1, bb1, 1, "A")
        with tc.tile_pool(name="pj1", bufs=2, space="PSUM", side="right") as pp:
            qT = proj_T(xn, "wq_s", "B", pp)
            kT = proj_T(xn, "wk_s", "D", pp)
            v = proj_nat(xn, "wv_s", "E", pp)
        cT = attention(qT, kT, v, cls_self, mtiles_s, "B")
        with tc.tile_pool(name="pj2", bufs=2, space="PSUM", side="right") as pp:
            out_proj_residual(cT, "wo_s", pp)

        # LN2 + cross-attention
        zn = layer_norm(xres, g2, bb2, 2, "A")
        with tc.tile_pool(name="pj3", bufs=2, space="PSUM", side="right") as pp:
            qTc = proj_T(zn, "wq_c", "B", pp)
            # encoder_output^T loads reuse group A (zn dead after qTc)
            enc = []
            for k in range(NT):
                t = big("A", k)
                nc.sync.dma_start(out=t, in_=encT_d.ap()[k * P:(k + 1) * P, :])
                enc.append(t)
            kTc = proj_T(enc, "wk_c", "D", pp)
            vc = proj_nat(enc, "wv_c", "E", pp)
        cTc = attention(qTc, kTc, vc, cls_cross, mtiles_c, "B")
        with tc.tile_pool(name="pj4", bufs=2, space="PSUM", side="right") as pp:
            out_proj_residual(cTc, "wo_c", pp)

        # LN3 + FFN
        fn = layer_norm(xres, g3, bb3, 3, "A")
        hgroups = (["B"] * 8 + ["D"] * 8 + ["E"] * 8 + ["C"] * 8)
        hT = []
        with tc.tile_pool(name="ffn1", bufs=2, space="PSUM", side="right") as pp:
            for quarter in range(4):
                w1t = []
                for k in range(NT):
                    t = wpool.tile([P, H], BF, tag=f"w{k}", name=f"w{k}")
                    nc.sync.dma_start(
                        out=t, in_=w1T_d.ap()[k * P:(k + 1) * P,
                                              quarter * H:(quarter + 1) * H])
                    w1t.append(t)
                for j in range(8):
                    m = quarter * 8 + j
                    d = big(hgroups[m], m % 8)
                    pss = [pp.tile([P, QW], F32, tag=f"pp{c}", name=f"pp{c}")
                           for c in range(NQ)]
                    for k in range(NT):
                        for c in range(NQ):
                            nc.tensor.matmul(
                                pss[c], lhsT=w1t[k][:, j * P:(j + 1) * P],
                                rhs=fn[k][:, c * QW:(c + 1) * QW],
                                start=(k == 0), stop=(k == NT - 1))
                    for c in range(NQ):
                        # h = relu(ps + b1)
                        nc.vector.tensor_scalar(
                            out=d[:, c * QW:(c + 1) * QW], in0=pss[c],
                            scalar1=b1[:, m:m + 1], scalar2=0.0,
                            op0=AOP.add, op1=AOP.max)
                    hT.append(d)

        with tc.tile_pool(name="ffn2", bufs=1, space="PSUM") as pp:
            for c in range(NQ):
                sl = slice(c * QW, (c + 1) * QW)
                accs = [pp.tile([P, QW], F32, tag=f"acc{m}", name=f"acc{m}") for m in range(NT)]
                for k2 in range(FF // P):
                    t = wpool.tile([P, H], BF, tag=f"w{k2 % 8}", name=f"w{k2 % 8}")
                    nc.sync.dma_start(
                        out=t, in_=w2T_d.ap()[k2 * P:(k2 + 1) * P, :])
                    for m in range(NT):
                        nc.tensor.matmul(
                            accs[m], lhsT=t[:, m * P:(m + 1) * P],
                            rhs=hT[k2][:, sl],
                            start=(k2 == 0), stop=(k2 == FF // P - 1))
                for m in range(NT):
                    so = stg.tile([P, QW], F32, tag="outst", name="outst")
                    nc.vector.scalar_tensor_tensor(
                        out=so, in0=accs[m], scalar=b2[:, m:m + 1],
                        in1=xres[m][:, sl], op0=AOP.add, op1=AOP.add)
                    nc.sync.dma_start(
                        out=outT_d.ap()[m * P:(m + 1) * P, sl], in_=so)


# ---------------------------------------------------------------------------
# host-side runner
# ---------------------------------------------------------------------------

class _Runner:
    """Cached jax-jitted 8-core runner for a compiled Bass module (mirrors
    concourse.bass2jax.run_bass_via_pjrt, but reusable across calls and with
    input staging separated from execution for timing)."""

    def __init__(self, nc):
        import jax
        from jax.sharding import Mesh, PartitionSpec, NamedSharding
        from jax.experimental.shard_map import shard_map
        from concourse import bass2jax, mybir as _mybir

        bass2jax.install_neuronx_cc_hook()
        self._jax = jax

        partition_name = (nc.partition_id_tensor.name
                          if nc.partition_id_tensor else None)
        in_names, out_names, out_avals, zero_shapes = [], [], [], []
        for alloc in nc.m.functions[0].allocations:
            if not isinstance(alloc, _mybir.MemoryLocationSet):
                continue
            name = alloc.memorylocations[0].name
            if alloc.kind == "ExternalInput":
                if name != partition_name:
                    in_names.append(name)
            elif alloc.kind == "ExternalOutput":
                out_names.append(name)
                shape = tuple(alloc.tensor_shape)
                dtype = _mybir.dt.np(alloc.dtype)
                out_avals.append(jax.core.ShapedArray(shape, dtype))
                zero_shapes.append((shape, dtype))
        self.in_names = in_names
        self.out_names = out_names
        self.out_avals = out_avals
        self.zero_shapes = zero_shapes
        n_params, n_outs = len(in_names), len(out_avals)
        all_in_names = in_names + out_names
        if partition_name is not None:
            all_in_names = all_in_names + [partition_name]
        donate = tuple(range(n_params, n_params + n_outs))

        def _body(*args):
            operands = list(args)
            if partition_name is not None:
                operands.append(bass2jax.partition_id_tensor())
            outs = bass2jax._bass_exec_p.bind(
                *operands,
                out_avals=tuple(out_avals),
                in_names=tuple(all_in_names),
                out_names=tuple(out_names),
                lowering_input_output_aliases=(),
                sim_require_finite=True,
                sim_require_nnan=True,
                nc=nc,
            )
            return tuple(outs)

        devices = jax.devices()[:NCORES]
        mesh = Mesh(np.asarray(devices), ("core",))
        self.sharding = NamedSharding(mesh, PartitionSpec("core"))
        in_specs = (PartitionSpec("core"),) * (n_params + n_outs)
        out_specs = (PartitionSpec("core"),) * n_outs
        self.sharded = jax.jit(
            shard_map(_body, mesh=mesh, in_specs=in_specs,
                      out_specs=out_specs, check_rep=False),
            donate_argnums=donate, keep_unused=True)

    def put(self, in_maps):
        """Stage concatenated per-core inputs onto the devices."""
        concat = [
            np.concatenate([np.asarray(in_maps[c][nm]) for c in range(NCORES)],
                           axis=0)
            for nm in self.in_names
        ]
        return [self._jax.device_put(a, self.sharding) for a in concat]

    def zeros(self):
        return [
            self._jax.device_put(
                np.zeros((NCORES * s[0], *s[1:]), d), self.sharding)
            for (s, d) in self.zero_shapes
        ]

    def exec(self, args, zeros):
        out = self.sharded(*args, *zeros)
        self._jax.block_until_ready(out)
        return out

    def __call__(self, in_maps):
        out_arrs = [np.asarray(a) for a in self.exec(self.put(in_maps),
                                                     self.zeros())]
        return [
            {nm: out_arrs[i].reshape(NCORES, *self.out_avals[i].shape)[c]
             for i, nm in enumerate(self.out_names)}
            for c in range(NCORES)
        ]


def _make_runner(nc):
    return _Runner(nc)


def _prep_inputs(input_, encoder_output, self_attn_mask, attn_mask,
                 Wq_s, Wk_s, Wv_s, Wo_s, Wq_c, Wk_c, Wv_c, Wo_c,
                 w1, b1, w2, b2, g_mmha, b_mmha, g_mha, b_mha, g_ffn, b_ffn):
    def bfT(a):
        return np.ascontiguousarray(np.asarray(a, np.float32).T).astype(BF16)

    shared = {
        "wq_s": bfT(Wq_s), "wk_s": bfT(Wk_s), "wv_s": bfT(Wv_s),
        "wo_s": bfT(Wo_s), "wq_c": bfT(Wq_c), "wk_c": bfT(Wk_c),
        "wv_c": bfT(Wv_c), "wo_c": bfT(Wo_c),
        "w1T": bfT(w1), "w2T": bfT(w2),
        "b1": np.asarray(b1, np.float32), "b2": np.asarray(b2, np.float32),
        "g1": np.asarray(g_mmha, np.float32), "bb1": np.asarray(b_mmha, np.float32),
        "g2": np.asarray(g_mha, np.float32), "bb2": np.asarray(b_mha, np.float32),
        "g3": np.asarray(g_ffn, np.float32), "bb3": np.asarray(b_ffn, np.float32),
    }
    m_s = np.asarray(self_attn_mask, bool)
    m_c = np.asarray(attn_mask, bool)

    def canon_map(mask, cls):
        """Map each MASKED (kb, qc) block to a canonical key; blocks with
        identical content (across the whole batch) share a key/SBUF tile."""
        canon = {}
        seen = {}
        for kb in range(NTK):
            for qc in range(NQ):
                if cls[kb, qc] != MASKED:
                    continue
                blk = mask[:, qc * QW:(qc + 1) * QW, kb * P:(kb + 1) * P]
                hkey = hash(blk.tobytes())
                if hkey not in seen:
                    seen[hkey] = f"{kb}_{qc}"
                canon[(kb, qc)] = seen[hkey]
        return canon
    in_maps = []
    for b in range(B):
        im = dict(shared)
        im["xT"] = np.ascontiguousarray(np.asarray(input_[b], np.float32).T)
        im["encT"] = np.ascontiguousarray(
            np.asarray(encoder_output[b], np.float32).T).astype(BF16)
        im["mm_s"] = np.ascontiguousarray(
            (~m_s[b]).T.astype(np.float32)).astype(BF16)
        im["mm_c"] = np.ascontiguousarray(
            (~m_c[b]).T.astype(np.float32)).astype(BF16)
        in_maps.append(im)
    cls_s, cls_c = _classify(m_s), _classify(m_c)
    return in_maps, cls_s, cls_c, canon_map(m_s, cls_s), canon_map(m_c, cls_c)


def kernel(**inputs):
    in_maps, cls_s, cls_c, canon_s, canon_c = _prep_inputs(**inputs)
    key = (cls_s.tobytes(), cls_c.tobytes(),
           tuple(sorted(canon_s.items())), tuple(sorted(canon_c.items())))
    if key not in _cache:
        nc = _build(cls_s, cls_c, canon_s, canon_c)
        _cache[key] = _make_runner(nc)
    results = _cache[key](in_maps)
    out = np.empty((B, T, H), np.float32)
    for b in range(B):
        out[b] = results[b]["outT"].T
    return out



# revision 3
# speedup vs baseline: 146.1009x; 146.1009x over previous
"""Trainium2 Bass kernel for nn_DecoderBlock (B=8, T=TE=1024, H=1024, NH=8).

Strategy: pure data-parallel over batch — batch element b runs on NeuronCore b,
no collectives. All on-chip compute is done in transposed layout [feature,
token] so no on-chip transposes are ever needed:
  - host pre-transposes input_/encoder_output and all weight matrices
  - layernorm stats (sums over the feature axis = partition axis) via
    ones-vector matmuls on the PE; affine params become per-partition scalars
  - attention computes scores transposed (s^T[k,q] = K^T-block^T... i.e.
    lhsT=K^T, rhs=Q^T), softmax denominator is folded in after the context
    matmul (exp without max-subtraction is safe: |scores| <~ 8 here)
  - causal-mask blocks that are fully masked are skipped entirely; partially
    masked blocks multiply exp(s) by a 0/1 mask tile loaded from the host
Matmuls in bf16 with fp32 PSUM accumulation; residual stream kept in fp32.
"""

import sys

for _p in ("/opt/trn_rl_repo", "/root/.axon_site/_ro/trn_rl_repo"):
    if _p not in sys.path:
        sys.path.append(_p)

import numpy as np
import ml_dtypes

import concourse.bass as bass
import concourse.mybir as mybir
import concourse.tile as tile
from concourse import bacc

BF16 = ml_dtypes.bfloat16
F32 = mybir.dt.float32
F16 = mybir.dt.float16
BF = mybir.dt.bfloat16

B = 8
T = 1024
TE = 1024
H = 1024
NH = 8
DK = H // NH  # 128
FF = 4 * H
P = 128
NT = H // P       # 8 feature blocks
NTK = T // P      # 8 key blocks
NQ = 2            # token chunks
QW = T // NQ      # 512
NCORES = 8
EPS = 1e-5
ISCALE = float(1.0 / np.sqrt(DK))

FULL, MASKED, SKIP = 0, 1, 2

AOP = mybir.AluOpType
AF = mybir.ActivationFunctionType

_cache = {}


def _classify(mask):
    """mask: [B, TQ, TK] bool (True = masked out). Block structure over
    (k_block, q_chunk), unioned across batch so one NEFF serves all cores."""
    cls = np.zeros((NTK, NQ), np.int32)
    for kb in range(NTK):
        for qc in range(NQ):
            blk = mask[:, qc * QW:(qc + 1) * QW, kb * P:(kb + 1) * P]
            if blk.all():
                cls[kb, qc] = SKIP
            elif blk.any():
                cls[kb, qc] = MASKED
            else:
                cls[kb, qc] = FULL
    return cls


def _build(cls_self, cls_cross, canon_s=None, canon_c=None, reps=1,
           loop_reps=1):
    nc = bacc.Bacc("TRN2", target_bir_lowering=False, debug=False,
                   num_devices=NCORES)

    xT_d = nc.dram_tensor("xT", [H, T], F32, kind="ExternalInput")
    encT_d = nc.dram_tensor("encT", [H, TE], BF, kind="ExternalInput")
    mm_s_d = nc.dram_tensor("mm_s", [T, T], BF, kind="ExternalInput")
    mm_c_d = nc.dram_tensor("mm_c", [TE, T], BF, kind="ExternalInput")
    wd = {}
    for nm in ("wq_s", "wk_s", "wv_s", "wo_s", "wq_c", "wk_c", "wv_c", "wo_c"):
        wd[nm] = nc.dram_tensor(nm, [H, H], BF, kind="ExternalInput")
    w1T_d = nc.dram_tensor("w1T", [H, FF], BF, kind="ExternalInput")
    w2T_d = nc.dram_tensor("w2T", [FF, H], BF, kind="ExternalInput")
    vd = {}
    vd["b1"] = nc.dram_tensor("b1", [FF], F32, kind="ExternalInput")
    for nm in ("b2", "g1", "bb1", "g2", "bb2", "g3", "bb3"):
        vd[nm] = nc.dram_tensor(nm, [H], F32, kind="ExternalInput")
    outT_d = nc.dram_tensor("outT", [H, T], F32, kind="ExternalOutput")

    with tile.TileContext(nc) as tc:
        if loop_reps > 1:
            # hardware loop re-executing the (idempotent) block body; used
            # for launch-overhead-amortized HW timing
            with tc.For_i(0, loop_reps):
                _emit(nc, tc, cls_self, cls_cross, canon_s, canon_c,
                      xT_d, encT_d, mm_s_d, mm_c_d,
                      wd, w1T_d, w2T_d, vd, outT_d)
        else:
            for _ in range(reps):
                _emit(nc, tc, cls_self, cls_cross, canon_s, canon_c,
                      xT_d, encT_d, mm_s_d, mm_c_d,
                      wd, w1T_d, w2T_d, vd, outT_d)
    nc.compile()
    return nc


def _emit(nc, tc, cls_self, cls_cross, canon_s, canon_c,
          xT_d, encT_d, mm_s_d, mm_c_d,
          wd, w1T_d, w2T_d, vd, outT_d):

    def canon_key_fn(dname, kb, qc):
        cmap = canon_s if dname == "mm_s" else canon_c
        if cmap is None:
            return f"{kb}_{qc}"
        return cmap[(kb, qc)]
    import contextlib
    ctx = contextlib.ExitStack()
    with ctx:
        # f16 is used only for softmax-denominator / LN-stat broadcast
        # intermediates where ~5e-4 relative error is acceptable by design.
        ctx.enter_context(nc.allow_low_precision(
            reason="f16 broadcast/denominator intermediates"))
        persist = ctx.enter_context(tc.tile_pool(name="persist", bufs=1))
        bigs = ctx.enter_context(tc.tile_pool(name="bigs", bufs=1))
        wpool = ctx.enter_context(tc.tile_pool(name="wpool", bufs=2))
        epool = ctx.enter_context(tc.tile_pool(name="epool", bufs=2))
        accp = ctx.enter_context(tc.tile_pool(name="accp", bufs=2))
        tmpp = ctx.enter_context(tc.tile_pool(name="tmpp", bufs=2))
        smp = ctx.enter_context(tc.tile_pool(name="smp", bufs=1))
        rdp = ctx.enter_context(tc.tile_pool(name="rdp", bufs=2))
        stg = ctx.enter_context(tc.tile_pool(name="stg", bufs=2))

        # ---- constants / params ----
        ones_k = persist.tile([P, 1], F32, tag="ones_k", name="ones_k")
        nc.vector.memset(ones_k, 1.0)
        ones_kb = persist.tile([P, 1], BF, tag="ones_kb", name="ones_kb")
        nc.vector.memset(ones_kb, 1.0)
        ones_k16 = persist.tile([P, 1], F16, tag="ones_k16", name="ones_k16")
        nc.vector.memset(ones_k16, 1.0)
        ones_r16 = persist.tile([1, P], F16, tag="ones_r16", name="ones_r16")
        nc.vector.memset(ones_r16, 1.0)
        ones_r = persist.tile([1, P], F32, tag="ones_r", name="ones_r")
        nc.vector.memset(ones_r, 1.0)
        eps_t = persist.tile([1, 1], F32, tag="eps", name="eps")
        nc.vector.memset(eps_t, EPS)

        # ---- residual stream x^T in fp32 ----
        xres = []
        for k in range(NT):
            t = persist.tile([P, T], F32, tag=f"xres{k}", name=f"xres{k}")
            xres.append(t)
        for c in range(NQ):
            for k in range(NT):
                nc.sync.dma_start(
                    out=xres[k][:, c * QW:(c + 1) * QW],
                    in_=xT_d.ap()[k * P:(k + 1) * P, c * QW:(c + 1) * QW])

        def load_vec(name, n):
            t = persist.tile([P, n // P], F32, tag=f"v_{name}", name=f"v_{name}")
            nc.sync.dma_start(out=t, in_=vd[name].ap().rearrange(
                "(n p) -> p n", p=P))
            return t

        g1 = load_vec("g1", H); bb1 = load_vec("bb1", H)
        g2 = load_vec("g2", H); bb2 = load_vec("bb2", H)
        g3 = load_vec("g3", H); bb3 = load_vec("bb3", H)
        b1 = load_vec("b1", FF); b2 = load_vec("b2", H)

        # mask multiplier tiles for partially-masked blocks; blocks whose
        # content is identical across (kb, qc) (e.g. causal diagonals) share
        # one SBUF tile, keyed by the canonical block in cls (negative codes).
        mtiles_s, mtiles_c = {}, {}
        for (cls, dram, store) in ((cls_self, mm_s_d, mtiles_s),
                                   (cls_cross, mm_c_d, mtiles_c)):
            canon = {}
            for kb in range(NTK):
                for qc in range(NQ):
                    if cls[kb, qc] != MASKED:
                        continue
                    key = canon_key_fn(dram.name, kb, qc)
                    if key not in canon:
                        mt = persist.tile([P, QW], BF,
                                          tag=f"msk_{dram.name}_{key}",
                                          name=f"msk_{dram.name}_{key}")
                        nc.sync.dma_start(
                            out=mt,
                            in_=dram.ap()[kb * P:(kb + 1) * P,
                                          qc * QW:(qc + 1) * QW])
                        canon[key] = mt
                    store[(kb, qc)] = canon[key]

        # big bf16 [P, T] tile groups (tags only; allocation at write time)
        def big(group, j):
            return bigs.tile([P, T], BF, tag=f"big{group}{j}", name=f"big{group}{j}")

        # ---------- helpers ----------
        def layer_norm(src_tiles, g, bb, gidx, dst_group):
            """src: 8 fp32 [P,T] tiles; returns 8 bf16 [P,T] tiles (dst_group)."""
            dst = [None] * NT
            with tc.tile_pool(name=f"ln{gidx}", bufs=1, space="PSUM", side="left") as pp:
                for c in range(NQ):
                    sl = slice(c * QW, (c + 1) * QW)
                    ps_sx = pp.tile([1, QW], F32, tag="sx", name="sx")
                    ps_sq = pp.tile([1, QW], F32, tag="sq", name="sq")
                    for k in range(NT):
                        xb = stg.tile([P, QW], BF, tag="xb", name="xb")
                        nc.vector.tensor_copy(out=xb, in_=src_tiles[k][:, sl])
                        sq = stg.tile([P, QW], BF, tag="sqt", name="sqt")
                        nc.vector.tensor_mul(out=sq, in0=xb, in1=xb)
                        nc.tensor.matmul(ps_sx, lhsT=ones_kb, rhs=xb,
                                         start=(k == 0), stop=(k == NT - 1))
                        nc.tensor.matmul(ps_sq, lhsT=ones_kb, rhs=sq,
                                         start=(k == 0), stop=(k == NT - 1))
                    mu = smp.tile([1, QW], F16, tag="mu", name="mu")
                    m2 = smp.tile([1, QW], F32, tag="m2", name="m2")
                    rs = smp.tile([1, QW], F16, tag="rs", name="rs")
                    nc.scalar.mul(out=mu, in_=ps_sx, mul=1.0 / H)
                    nc.scalar.mul(out=m2, in_=ps_sq, mul=1.0 / H)
                    # rs doubles as mu^2 scratch before holding 1/std
                    nc.vector.tensor_mul(out=rs, in0=mu, in1=mu)
                    nc.vector.tensor_sub(out=m2, in0=m2, in1=rs)
                    # m2 := sqrt(var + eps)
                    nc.scalar.activation(out=m2, in_=m2, func=AF.Sqrt,
                                         bias=eps_t)
                    nc.vector.reciprocal(out=rs, in_=m2)
                    ps_bm = pp.tile([P, QW], F32, tag="bm", name="bm")
                    ps_br = pp.tile([P, QW], F32, tag="br", name="br")
                    nc.tensor.matmul(ps_bm, lhsT=ones_r16, rhs=mu,
                                     start=True, stop=True)
                    nc.tensor.matmul(ps_br, lhsT=ones_r16, rhs=rs,
                                     start=True, stop=True)
                    # broadcasts to SBUF once per chunk so the per-tile DVE
                    # ops run in 2x mode (PSUM operands force 1x)
                    bm = tmpp.tile([P, QW], F32, tag="bm_sb", name="bm_sb", bufs=1)
                    nc.vector.tensor_copy(out=bm, in_=ps_bm)
                    br = tmpp.tile([P, QW], F32, tag="br_sb", name="br_sb", bufs=1)
                    nc.vector.tensor_copy(out=br, in_=ps_br)
                    for k in range(NT):
                        if dst[k] is None and c == 0:
                            dst[k] = big(dst_group, k)
                        tmp = tmpp.tile([P, QW], F32, tag="lnt", name="lnt")
                        nc.vector.tensor_sub(out=tmp, in0=src_tiles[k][:, sl],
                                             in1=bm)
                        nc.vector.tensor_mul(out=tmp, in0=tmp, in1=br)
                        nc.vector.tensor_scalar(
                            out=dst[k][:, sl], in0=tmp,
                            scalar1=g[:, k:k + 1], scalar2=bb[:, k:k + 1],
                            op0=AOP.mult, op1=AOP.add)
            return dst

        def load_w(dram, colsl=None, tag_suffix=""):
            """Load [H or P*8, 1024-wide] weight block-rows into wpool tags."""
            tiles = []
            for k in range(NT):
                t = wpool.tile([P, H], BF, tag=f"w{k}", name=f"w{k}")
                src = dram.ap()[k * P:(k + 1) * P, :] if colsl is None else \
                    dram.ap()[k * P:(k + 1) * P, colsl]
                nc.sync.dma_start(out=t, in_=src)
                tiles.append(t)
            return tiles

        def proj_T(src_tiles, wname, dst_group, pp):
            """dst^T[m][:,c] = sum_k W^T[k,m-block]^T... out = W @ src^T.
            Returns 8 bf16 [P,T] tiles."""
            wt = load_w(wd[wname])
            dst = []
            for m in range(NT):
                d = big(dst_group, m)
                pss = [pp.tile([P, QW], F32, tag=f"pp{c}", name=f"pp{c}")
                       for c in range(NQ)]
                for k in range(NT):
                    for c in range(NQ):
                        nc.tensor.matmul(pss[c],
                                         lhsT=wt[k][:, m * P:(m + 1) * P],
                                         rhs=src_tiles[k][:, c * QW:(c + 1) * QW],
                                         start=(k == 0), stop=(k == NT - 1))
                for c in range(NQ):
                    nc.scalar.copy(out=d[:, c * QW:(c + 1) * QW], in_=pss[c])
                dst.append(d)
            return dst

        def proj_nat(src_tiles, wname, dst_group, pp):
            """V = src @ W.T in natural [token, feature] layout."""
            wt = load_w(wd[wname])
            dst = []
            for tb in range(NT):
                d = big(dst_group, tb)
                pss = [pp.tile([P, QW], F32, tag=f"pp{c}", name=f"pp{c}")
                       for c in range(NQ)]
                for k in range(NT):
                    for c in range(NQ):
                        nc.tensor.matmul(pss[c],
                                         lhsT=src_tiles[k][:, tb * P:(tb + 1) * P],
                                         rhs=wt[k][:, c * QW:(c + 1) * QW],
                                         start=(k == 0), stop=(k == NT - 1))
                for c in range(NQ):
                    nc.scalar.copy(out=d[:, c * QW:(c + 1) * QW], in_=pss[c])
                dst.append(d)
            return dst

        def attention(qT, kT, v, cls, mtiles, dst_group):
            """qT,kT: 8 [P(d),T] bf16 tiles (tile h = head h); v: 8 [P(t),H]
            bf16 tiles. Returns c^T as 8 bf16 [P,T] tiles (tile h = head h).

            Software-pipelined over (head, chunk) units: unit i+1's scores
            matmuls are emitted before unit i's den/bcast/ctx matmuls so the
            PE has work while unit i's softmax (ACT exp + DVE tree) runs."""
            cT = {}
            units = [(h, qc) for h in range(NH) for qc in range(NQ)]

            def stage1(i, h, qc, pp):
                """paired scores -> exp -> masked mul -> denominator reduce."""
                qsl = slice(qc * QW, (qc + 1) * QW)
                kbs = [kb for kb in range(NTK) if cls[kb, qc] != SKIP]
                n = len(kbs)
                eall = epool.tile([P, NTK, QW], BF, tag="eall", name="eall")
                idx = 0
                pi = 0
                while idx < n:
                    m = min(2, n - idx)
                    ps = pp.tile([P, 2 * QW], F32, tag=f"s{pi % 2}",
                                 name=f"s{pi % 2}")
                    for j in range(m):
                        kb = kbs[idx + j]
                        nc.tensor.matmul(
                            ps[:, j * QW:(j + 1) * QW],
                            lhsT=kT[h][:, kb * P:(kb + 1) * P],
                            rhs=qT[h][:, qsl], start=True, stop=True)
                    nc.scalar.activation(
                        out=eall[:, idx:idx + m, :].rearrange("p a b -> p (a b)"),
                        in_=ps[:, 0:m * QW], func=AF.Exp, scale=ISCALE)
                    for j in range(m):
                        kb = kbs[idx + j]
                        if cls[kb, qc] == MASKED:
                            nc.vector.tensor_mul(
                                out=eall[:, idx + j, :], in0=eall[:, idx + j, :],
                                in1=mtiles[(kb, qc)])
                    idx += m
                    pi += 1
                # denominator: sum exp-pair outputs progressively so the
                # tree starts after the SECOND exp, not the last one. Fast
                # path for the even pair counts that actually occur (n=4,8);
                # generic fold otherwise.
                acc = accp.tile([P, QW], F16, tag="acc", name="acc")
                def flat(ap):
                    return ap.rearrange("p a b -> p (a b)")
                if n == 8:
                    pA = accp.tile([P, 2, QW], F16, tag="pA", name="pA")
                    nc.vector.tensor_add(out=flat(pA), in0=flat(eall[:, 0:2, :]),
                                         in1=flat(eall[:, 2:4, :]))
                    pB = accp.tile([P, 2, QW], F16, tag="pB", name="pB")
                    nc.vector.tensor_add(out=flat(pB), in0=flat(eall[:, 4:6, :]),
                                         in1=flat(eall[:, 6:8, :]))
                    nc.vector.tensor_add(out=pA[:, 0, :], in0=pA[:, 0, :],
                                         in1=pA[:, 1, :])
                    nc.vector.tensor_add(out=pB[:, 0, :], in0=pB[:, 0, :],
                                         in1=pB[:, 1, :])
                    nc.vector.tensor_add(out=acc, in0=pA[:, 0, :],
                                         in1=pB[:, 0, :])
                elif n == 4:
                    pA = accp.tile([P, 2, QW], F16, tag="pA", name="pA")
                    nc.vector.tensor_add(out=flat(pA), in0=flat(eall[:, 0:2, :]),
                                         in1=flat(eall[:, 2:4, :]))
                    nc.vector.tensor_add(out=acc, in0=pA[:, 0, :],
                                         in1=pA[:, 1, :])
                else:
                    # generic fold for arbitrary mask structures
                    m = n // 2
                    if m == 1:
                        nc.vector.tensor_add(out=acc, in0=eall[:, 0, :],
                                             in1=eall[:, 1, :])
                        if n % 2:
                            nc.vector.tensor_add(out=acc, in0=acc,
                                                 in1=eall[:, n - 1, :])
                        return kbs, eall, acc
                    a4 = accp.tile([P, NTK // 2, QW], F16, tag="a4",
                                   name="a4", bufs=1)
                    nc.vector.tensor_add(
                        out=flat(a4[:, 0:m, :]), in0=flat(eall[:, 0:m, :]),
                        in1=flat(eall[:, m:2 * m, :]))
                    if n % 2:
                        nc.vector.tensor_add(out=a4[:, 0, :], in0=a4[:, 0, :],
                                             in1=eall[:, n - 1, :])
                    while m > 2:
                        h2 = m // 2
                        nc.vector.tensor_add(
                            out=flat(a4[:, 0:h2, :]), in0=flat(a4[:, 0:h2, :]),
                            in1=flat(a4[:, h2:2 * h2, :]))
                        if m % 2:
                            nc.vector.tensor_add(out=a4[:, 0, :],
                                                 in0=a4[:, 0, :],
                                                 in1=a4[:, m - 1, :])
                        m = h2
                    nc.vector.tensor_add(out=acc, in0=a4[:, 0, :],
                                         in1=a4[:, 1, :])
                return kbs, eall, acc
                a4 = accp.tile([P, NTK // 2, QW], F16, tag="a4", name="a4",
                               bufs=1)
                nc.vector.tensor_add(
                    out=a4[:, 0:m, :].rearrange("p a b -> p (a b)"),
                    in0=eall[:, 0:m, :].rearrange("p a b -> p (a b)"),
                    in1=eall[:, m:2 * m, :].rearrange("p a b -> p (a b)"))
                if n % 2:
                    nc.vector.tensor_add(out=a4[:, 0, :], in0=a4[:, 0, :],
                                         in1=eall[:, n - 1, :])
                while m > 2:
                    h2 = m // 2
                    nc.vector.tensor_add(
                        out=a4[:, 0:h2, :].rearrange("p a b -> p (a b)"),
                        in0=a4[:, 0:h2, :].rearrange("p a b -> p (a b)"),
                        in1=a4[:, h2:2 * h2, :].rearrange("p a b -> p (a b)"))
                    if m % 2:
                        nc.vector.tensor_add(out=a4[:, 0, :], in0=a4[:, 0, :],
                                             in1=a4[:, m - 1, :])
                    m = h2
                nc.vector.tensor_add(out=acc, in0=a4[:, 0, :], in1=a4[:, 1, :])
                return kbs, eall, acc

            def stage2(i, h, qc, kbs, eall, acc, pp):
                """den matmul -> recip -> bcast -> ctx -> cT mul."""
                qsl = slice(qc * QW, (qc + 1) * QW)
                # ctx matmuls first: they need only the e tiles, which are
                # ready well before the denominator tree finishes
                ps_u = pp.tile([P, QW], F32, tag=f"u{qc % 2}",
                               name=f"u{qc % 2}")
                for j, kb in enumerate(kbs):
                    nc.tensor.matmul(
                        ps_u, lhsT=v[kb][:, h * P:(h + 1) * P],
                        rhs=eall[:, j, :],
                        start=(j == 0), stop=(j == len(kbs) - 1))
                ps_den = pp.tile([1, QW], F32, tag=f"db{i % 2}",
                                 name=f"db{i % 2}")
                nc.tensor.matmul(ps_den, lhsT=ones_k16, rhs=acc,
                                 start=True, stop=True)
                rden = rdp.tile([1, QW], F16, tag="rden", name="rden")
                nc.vector.reciprocal(out=rden, in_=ps_den)
                ps_bc = pp.tile([P, QW], F32, tag=f"db{i % 2}",
                                name=f"db{i % 2}b")
                nc.tensor.matmul(ps_bc, lhsT=ones_r16, rhs=rden,
                                 start=True, stop=True)
                brden = rdp.tile([P, QW], F32, tag="brden", name="brden")
                nc.vector.tensor_copy(out=brden, in_=ps_bc)
                if h not in cT:
                    cT[h] = big(dst_group, h)
                nc.vector.tensor_mul(out=cT[h][:, qsl], in0=ps_u, in1=brden)

            with tc.tile_pool(name=f"att{dst_group}", bufs=1,
                              space="PSUM") as pp:
                pending = []
                for i, (h, qc) in enumerate(units):
                    pending.append((i, h, qc) + stage1(i, h, qc, pp))
                    if len(pending) > 1:
                        stage2(*pending.pop(0), pp)
                for item in pending:
                    stage2(*item, pp)
            return [cT[h] for h in range(NH)]

        def out_proj_residual(cT, wname, pp):
            wt = load_w(wd[wname])
            for m in range(NT):
                pss = [pp.tile([P, QW], F32, tag=f"pp{c}", name=f"pp{c}")
                       for c in range(NQ)]
                for k in range(NT):
                    for c in range(NQ):
                        nc.tensor.matmul(pss[c],
                                         lhsT=wt[k][:, m * P:(m + 1) * P],
                                         rhs=cT[k][:, c * QW:(c + 1) * QW],
                                         start=(k == 0), stop=(k == NT - 1))
                for c in range(NQ):
                    sl = slice(c * QW, (c + 1) * QW)
                    nc.vector.tensor_add(out=xres[m][:, sl],
                                         in0=xres[m][:, sl], in1=pss[c])

        # ================= phases =================
        # LN1 + self-attention
        xn = layer_norm(xres, g1, bb1, 1, "A")
        with tc.tile_pool(name="pj1", bufs=2, space="PSUM", side="right") as pp:
            qT = proj_T(xn, "wq_s", "B", pp)
            kT = proj_T(xn, "wk_s", "D", pp)
            v = proj_nat(xn, "wv_s", "E", pp)
        cT = attention(qT, kT, v, cls_self, mtiles_s, "B")
        with tc.tile_pool(name="pj2", bufs=2, space="PSUM", side="right") as pp:
            out_proj_residual(cT, "wo_s", pp)

        # LN2 + cross-attention
        zn = layer_norm(xres, g2, bb2, 2, "A")
        with tc.tile_pool(name="pj3", bufs=2, space="PSUM", side="right") as pp:
            qTc = proj_T(zn, "wq_c", "B", pp)
            # encoder_output^T loads reuse group A (zn dead after qTc)
            enc = []
            for k in range(NT):
                t = big("A", k)
                nc.sync.dma_start(out=t, in_=encT_d.ap()[k * P:(k + 1) * P, :])
                enc.append(t)
            kTc = proj_T(enc, "wk_c", "D", pp)
            vc = proj_nat(enc, "wv_c", "E", pp)
        cTc = attention(qTc, kTc, vc, cls_cross, mtiles_c, "B")
        with tc.tile_pool(name="pj4", bufs=2, space="PSUM", side="right") as pp:
            out_proj_residual(cTc, "wo_c", pp)

        # LN3 + FFN
        fn = layer_norm(xres, g3, bb3, 3, "A")
        hgroups = (["B"] * 8 + ["D"] * 8 + ["E"] * 8 + ["C"] * 8)
        hT = []
        with tc.tile_pool(name="ffn1", bufs=2, space="PSUM", side="right") as pp:
            for quarter in range(4):
                w1t = []
                for k in range(NT):
                    t = wpool.tile([P, H], BF, tag=f"w{k}", name=f"w{k}")
                    nc.sync.dma_start(
                        out=t, in_=w1T_d.ap()[k * P:(k + 1) * P,
                                              quarter * H:(quarter + 1) * H])
                    w1t.append(t)
                for j in range(8):
                    m = quarter * 8 + j
                    d = big(hgroups[m], m % 8)
                    pss = [pp.tile([P, QW], F32, tag=f"pp{c}", name=f"pp{c}")
                           for c in range(NQ)]
                    for k in range(NT):
                        for c in range(NQ):
                            nc.tensor.matmul(
                                pss[c], lhsT=w1t[k][:, j * P:(j + 1) * P],
                                rhs=fn[k][:, c * QW:(c + 1) * QW],
                                start=(k == 0), stop=(k == NT - 1))
                    for c in range(NQ):
                        # h = relu(ps + b1)
                        nc.vector.tensor_scalar(
                            out=d[:, c * QW:(c + 1) * QW], in0=pss[c],
                            scalar1=b1[:, m:m + 1], scalar2=0.0,
                            op0=AOP.add, op1=AOP.max)
                    hT.append(d)

        with tc.tile_pool(name="ffn2", bufs=1, space="PSUM") as pp:
            for c in range(NQ):
                sl = slice(c * QW, (c + 1) * QW)
                accs = [pp.tile([P, QW], F32, tag=f"acc{m}", name=f"acc{m}") for m in range(NT)]
                for k2 in range(FF // P):
                    t = wpool.tile([P, H], BF, tag=f"w{k2 % 8}", name=f"w{k2 % 8}")
                    nc.sync.dma_start(
                        out=t, in_=w2T_d.ap()[k2 * P:(k2 + 1) * P, :])
                    for m in range(NT):
                        nc.tensor.matmul(
                            accs[m], lhsT=t[:, m * P:(m + 1) * P],
                            rhs=hT[k2][:, sl],
                            start=(k2 == 0), stop=(k2 == FF // P - 1))
                for m in range(NT):
                    so = stg.tile([P, QW], F32, tag="outst", name="outst")
                    nc.vector.scalar_tensor_tensor(
                        out=so, in0=accs[m], scalar=b2[:, m:m + 1],
                        in1=xres[m][:, sl], op0=AOP.add, op1=AOP.add)
                    nc.sync.dma_start(
                        out=outT_d.ap()[m * P:(m + 1) * P, sl], in_=so)


# ---------------------------------------------------------------------------
# host-side runner
# ---------------------------------------------------------------------------

class _Runner:
    """Cached jax-jitted 8-core runner for a compiled Bass module (mirrors
    concourse.bass2jax.run_bass_via_pjrt, but reusable across calls and with
    input staging separated from execution for timing)."""

    def __init__(self, nc):
        import jax
        from jax.sharding import Mesh, PartitionSpec, NamedSharding
        from jax.experimental.shard_map import shard_map
        from concourse import bass2jax, mybir as _mybir

        bass2jax.install_neuronx_cc_hook()
        self._jax = jax

        partition_name = (nc.partition_id_tensor.name
                          if nc.partition_id_tensor else None)
        in_names, out_names, out_avals, zero_shapes = [], [], [], []
        for alloc in nc.m.functions[0].allocations:
            if not isinstance(alloc, _mybir.MemoryLocationSet):
                continue
            name = alloc.memorylocations[0].name
            if alloc.kind == "ExternalInput":
                if name != partition_name:
                    in_names.append(name)
            elif alloc.kind == "ExternalOutput":
                out_names.append(name)
                shape = tuple(alloc.tensor_shape)
                dtype = _mybir.dt.np(alloc.dtype)
                out_avals.append(jax.core.ShapedArray(shape, dtype))
                zero_shapes.append((shape, dtype))
        self.in_names = in_names
        self.out_names = out_names
        self.out_avals = out_avals
        self.zero_shapes = zero_shapes
        n_params, n_outs = len(in_names), len(out_avals)
        all_in_names = in_names + out_names
        if partition_name is not None:
            all_in_names = all_in_names + [partition_name]
        donate = tuple(range(n_params, n_params + n_outs))

        def _body(*args):
            operands = list(args)
            if partition_name is not None:
                operands.append(bass2jax.partition_id_tensor())
            outs = bass2jax._bass_exec_p.bind(
                *operands,
                out_avals=tuple(out_avals),
                in_names=tuple(all_in_names),
                out_names=tuple(out_names),
                lowering_input_output_aliases=(),
                sim_require_finite=True,
                sim_require_nnan=True,
                nc=nc,
            )
            return tuple(outs)

        devices = jax.devices()[:NCORES]
        mesh = Mesh(np.asarray(devices), ("core",))
        self.sharding = NamedSharding(mesh, PartitionSpec("core"))
        in_specs = (PartitionSpec("core"),) * (n_params + n_outs)
        out_specs = (PartitionSpec("core"),) * n_outs
        self.sharded = jax.jit(
            shard_map(_body, mesh=mesh, in_specs=in_specs,
                      out_specs=out_specs, check_rep=False),
            donate_argnums=donate, keep_unused=True)

    def put(self, in_maps):
        """Stage concatenated per-core inputs onto the devices."""
        concat = [
            np.concatenate([np.asarray(in_maps[c][nm]) for c in range(NCORES)],
                           axis=0)
            for nm in self.in_names
        ]
        return [self._jax.device_put(a, self.sharding) for a in concat]

    def zeros(self):
        return [
            self._jax.device_put(
                np.zeros((NCORES * s[0], *s[1:]), d), self.sharding)
            for (s, d) in self.zero_shapes
        ]

    def exec(self, args, zeros):
        out = self.sharded(*args, *zeros)
        self._jax.block_until_ready(out)
        return out

    def __call__(self, in_maps):
        out_arrs = [np.asarray(a) for a in self.exec(self.put(in_maps),
                                                     self.zeros())]
        return [
            {nm: out_arrs[i].reshape(NCORES, *self.out_avals[i].shape)[c]
             for i, nm in enumerate(self.out_names)}
            for c in range(NCORES)
        ]


def _make_runner(nc):
    return _Runner(nc)


def _prep_inputs(input_, encoder_output, self_attn_mask, attn_mask,
                 Wq_s, Wk_s, Wv_s, Wo_s, Wq_c, Wk_c, Wv_c, Wo_c,
                 w1, b1, w2, b2, g_mmha, b_mmha, g_mha, b_mha, g_ffn, b_ffn):
    def bfT(a):
        return np.ascontiguousarray(np.asarray(a, np.float32).T).astype(BF16)

    shared = {
        "wq_s": bfT(Wq_s), "wk_s": bfT(Wk_s), "wv_s": bfT(Wv_s),
        "wo_s": bfT(Wo_s), "wq_c": bfT(Wq_c), "wk_c": bfT(Wk_c),
        "wv_c": bfT(Wv_c), "wo_c": bfT(Wo_c),
        "w1T": bfT(w1), "w2T": bfT(w2),
        "b1": np.asarray(b1, np.float32), "b2": np.asarray(b2, np.float32),
        "g1": np.asarray(g_mmha, np.float32), "bb1": np.asarray(b_mmha, np.float32),
        "g2": np.asarray(g_mha, np.float32), "bb2": np.asarray(b_mha, np.float32),
        "g3": np.asarray(g_ffn, np.float32), "bb3": np.asarray(b_ffn, np.float32),
    }
    m_s = np.asarray(self_attn_mask, bool)
    m_c = np.asarray(attn_mask, bool)

    def canon_map(mask, cls):
        """Map each MASKED (kb, qc) block to a canonical key; blocks with
        identical content (across the whole batch) share a key/SBUF tile."""
        canon = {}
        seen = {}
        for kb in range(NTK):
            for qc in range(NQ):
                if cls[kb, qc] != MASKED:
                    continue
                blk = mask[:, qc * QW:(qc + 1) * QW, kb * P:(kb + 1) * P]
                hkey = hash(blk.tobytes())
                if hkey not in seen:
                    seen[hkey] = f"{kb}_{qc}"
                canon[(kb, qc)] = seen[hkey]
        return canon
    in_maps = []
    for b in range(B):
        im = dict(shared)
        im["xT"] = np.ascontiguousarray(np.asarray(input_[b], np.float32).T)
        im["encT"] = np.ascontiguousarray(
            np.asarray(encoder_output[b], np.float32).T).astype(BF16)
        im["mm_s"] = np.ascontiguousarray(
            (~m_s[b]).T.astype(np.float32)).astype(BF16)
        im["mm_c"] = np.ascontiguousarray(
            (~m_c[b]).T.astype(np.float32)).astype(BF16)
        in_maps.append(im)
    cls_s, cls_c = _classify(m_s), _classify(m_c)
    return in_maps, cls_s, cls_c, canon_map(m_s, cls_s), canon_map(m_c, cls_c)


def kernel(**inputs):
    in_maps, cls_s, cls_c, canon_s, canon_c = _prep_inputs(**inputs)
    key = (cls_s.tobytes(), cls_c.tobytes(),
           tuple(sorted(canon_s.items())), tuple(sorted(canon_c.items())))
    if key not in _cache:
        nc = _build(cls_s, cls_c, canon_s, canon_c)
        _cache[key] = _make_runner(nc)
    results = _cache[key](in_maps)
    out = np.empty((B, T, H), np.float32)
    for b in range(B):
        out[b] = results[b]["outT"].T
    return out

